# revision 25
# baseline (speedup 1.0000x reference)
"""Distributed Trainium2 kernel for pre-LN multi-head self-attention.

Reference computation (n=2048, d=1024, 16 heads x 64):
    xn  = LayerNorm(x) * ln_scale + ln_bias
    qkv = xn @ w_qkv ; split -> q,k,v [16, 2048, 64]
    sim = (q @ k^T) * d**-0.5 ; attn = softmax(sim)
    out = concat_heads(attn @ v) @ w_out + b_out

Sharding: 2 heads per core (tensor parallel). Each core:
  - LayerNorm stats (replicated); xn^T is built by PE matmuls of x-tile
    chunks against diag(rstd) (transpose + rstd scale fused); the
    -mu*rstd term folds into QKV as rank-1 corrections
    (-colsum(W) (x) mu*rstd); ln_scale/ln_bias fold into weights/biases
    on the host
  - attention in transposed layout (keys on partitions); a ones column
    in v yields softmax denominators in PSUM row 64 for free
  - denominators: DVE copy -> gpsimd partition_broadcast -> 64-lane
    reciprocal -> normalize; one AllGather per 512-row stage (both
    heads packed [128, 512])
  - final projection slice accumulates +b_out via rank-1 and DMAs
    PSUM -> DRAM directly
Host assembles the 8 [128, 2048] outT shards into [2048, 1024].

Scheduling: attn@v trails sim by AV_LAG key chunks inside the same
stage, so each stage's AllGather issues right after the stage ends and
its projection lands inside the next stage. Dummy matmuls bridge PE
idle gaps (first DMA wait, g4 boundaries, ACT-bound attention stages)
to keep the HAM clock gate at full rate.
"""

import sys

import ml_dtypes
import numpy as np

for _p in ("/opt/trn_rl_repo", "/root/.axon_site/_ro/trn_rl_repo"):
    if _p not in sys.path:
        sys.path.append(_p)

N = 2048          # sequence length
D = 1024          # model dim
HEADS = 16
DH = 64
NCORES = 8
HL = HEADS // NCORES          # heads per core (2)
HC = HL * DH                  # head cols per core (128)
LN_EPS = 1e-6
SIM_SCALE = float(D) ** -0.5  # reference scales by input dim

P = 128
RT = N // P        # 16 row tiles
DC = D // P        # 8 dim chunks
RC_W = 512         # row-chunk width (one stage)
NRC = N // RC_W    # 4 stages
AV_LAG = 6         # attn@v trails sim by this many key chunks
DUM_W = 128        # dummy matmul width (HAM keep-warm)

MM_DT = "bf16"
DEBUG = False      # add intermediate-dump outputs to the graph

_BUILT = None


def _build():
    """Build the SPMD Bass graph (same graph on all 8 cores)."""
    from contextlib import ExitStack

    import concourse.tile as tile
    from concourse import bacc, mybir
    from concourse.masks import make_identity

    f32 = mybir.dt.float32
    dt_mm = {"f32": f32, "f32r": mybir.dt.float32r,
             "bf16": mybir.dt.bfloat16}[MM_DT]
    AF = mybir.ActivationFunctionType

    nc = bacc.Bacc(None, num_devices=NCORES)

    x_d = nc.declare_dram_parameter("x", [N, D], f32, isOutput=False)
    wq_d = nc.declare_dram_parameter("wq", [D, HC], dt_mm, isOutput=False)
    wk_d = nc.declare_dram_parameter("wk", [D, HC], dt_mm, isOutput=False)
    wv_d = nc.declare_dram_parameter("wv", [D, HC], dt_mm, isOutput=False)
    qb_d = nc.declare_dram_parameter("qb", [HC], f32, isOutput=False)
    kb_d = nc.declare_dram_parameter("kb", [HC], f32, isOutput=False)
    vb_d = nc.declare_dram_parameter("vb", [HC], f32, isOutput=False)
    qcs_d = nc.declare_dram_parameter("qcs", [HC], dt_mm, isOutput=False)
    kcs_d = nc.declare_dram_parameter("kcs", [HC], dt_mm, isOutput=False)
    vcs_d = nc.declare_dram_parameter("vcs", [HC], dt_mm, isOutput=False)
    wo_d = nc.declare_dram_parameter("wo", [D, HC], dt_mm, isOutput=False)
    bo_d = nc.declare_dram_parameter("bo", [HC], f32, isOutput=False)
    out_d = nc.declare_dram_parameter("out", [HC, N], f32, isOutput=True)
    if DEBUG:
        dbg_qt = nc.declare_dram_parameter("dbg_qt", [P, N], f32, isOutput=True)
        dbg_kt = nc.declare_dram_parameter("dbg_kt", [P, N], f32, isOutput=True)
        dbg_xnt = nc.declare_dram_parameter("dbg_xnt", [P, N], f32,
                                            isOutput=True)
        dbg_v = nc.declare_dram_parameter("dbg_v", [P, RT * HL * (DH + 1)],
                                          f32, isOutput=True)
        dbg_den = nc.declare_dram_parameter("dbg_den", [DH + 1, 2 * RC_W],
                                            f32, isOutput=True)
        dbg_attn = nc.declare_dram_parameter("dbg_attn", [DH, N], f32,
                                             isOutput=True)

    groups = [list(range(NCORES))]

    with ExitStack() as ctx:
        tc = ctx.enter_context(tile.TileContext(nc))

        dram = ctx.enter_context(tc.tile_pool(name="dram", bufs=1, space="DRAM"))
        ag_in = [dram.tile([P, RC_W], dt_mm, name=f"ag_in{i}")
                 for i in range(NRC)]
        ag_out = [dram.tile([NCORES * P, RC_W], dt_mm, addr_space="Shared",
                            name=f"ag_out{i}") for i in range(NRC)]

        singles = ctx.enter_context(tc.tile_pool(name="singles", bufs=1))
        xp = ctx.enter_context(tc.tile_pool(name="xp", bufs=3))
        xbp = ctx.enter_context(tc.tile_pool(name="xb", bufs=8))
        dump = ctx.enter_context(tc.tile_pool(name="dum", bufs=1, space="PSUM"))

        ident = singles.tile([P, P], dt_mm)
        make_identity(nc, ident)
        warm_rhs = singles.tile([P, RC_W], dt_mm)
        nc.vector.memset(warm_rhs, 0.0)
        eps_t = singles.tile([P, 1], f32)
        nc.vector.memset(eps_t, LN_EPS)
        ones_row = singles.tile([P, RC_W], dt_mm)
        nc.gpsimd.memset(ones_row, 1.0)

        dum_ps = dump.tile([P, RC_W], f32, tag="warm")

        def dummy_mms(n, width=DUM_W):
            for _ in range(n):
                nc.tensor.matmul(dum_ps[:, 0:width], ident,
                                 warm_rhs[:, 0:width], start=True, stop=True)

        # x tile DMAs first so tile 0's pipeline starts ASAP
        x_tiles = []
        for rt in range(3):
            x_t = xp.tile([P, D], f32, tag="x", name=f"x{rt}")
            nc.sync.dma_start(out=x_t, in_=x_d[rt * P:(rt + 1) * P, :])
            x_tiles.append(x_t)

        # weights / biases (wk first: K projection runs first)
        wq_sb = singles.tile([P, DC, HC], dt_mm)
        wk_sb = singles.tile([P, DC, HC], dt_mm)
        wv_sb = singles.tile([P, DC, HC], dt_mm)
        wo_sb = singles.tile([P, DC, HC], dt_mm)
        nc.sync.dma_start(
            out=wk_sb, in_=wk_d[:, :].rearrange("(c p) m -> p c m", p=P)
        )
        nc.sync.dma_start(
            out=wq_sb, in_=wq_d[:, :].rearrange("(c p) m -> p c m", p=P)
        )
        nc.sync.dma_start(
            out=wv_sb, in_=wv_d[:, :].rearrange("(c p) m -> p c m", p=P)
        )
        qb_t = singles.tile([P, 1], f32)
        kb_t = singles.tile([P, 1], f32)
        vb_t = singles.tile([P, 1], f32)
        bo_t = singles.tile([P, 1], f32)
        # -colsum rows, each on partition 0 (stationary base must be 0/32/64)
        cs_q = singles.tile([1, HC], dt_mm)
        cs_k = singles.tile([1, HC], dt_mm)
        cs_v = singles.tile([1, HC], dt_mm)
        for b_t, b_d in ((qb_t, qb_d), (kb_t, kb_d), (vb_t, vb_d), (bo_t, bo_d)):
            nc.sync.dma_start(out=b_t, in_=b_d[:].rearrange("(p o) -> p o", o=1))
        for cs_t, cs_d in ((cs_q, qcs_d), (cs_k, kcs_d), (cs_v, vcs_d)):
            nc.sync.dma_start(
                out=cs_t, in_=cs_d[:].rearrange("(o m) -> o m", o=1)
            )

        # long-lived activations
        qT = singles.tile([P, N], dt_mm)        # [2*64 qdims, rows]
        kT = singles.tile([P, N], dt_mm)
        v_sb = singles.tile([P, RT, HL, DH + 1], dt_mm)  # [key%128, kc, h, v|1]
        attn_h = [singles.tile([DH, N], dt_mm, name=f"attn_h{h}")
                  for h in range(HL)]
        mustdT0 = singles.tile([1, N], dt_mm)    # (mu*rstd) as one row, part. 0
        bo_row = singles.tile([4, P], dt_mm)     # row 0 = b_out slice as a row

        nc.gpsimd.memset(v_sb[:, :, :, DH:], 1.0)  # ones column

        # ---- stages A-C: LayerNorm -> xn^T (rstd folded) -> q/k/v ----------
        with (
            tc.tile_pool(name="stat", bufs=4) as statp,
            tc.tile_pool(name="tp", bufs=2, space="PSUM") as tp,
            tc.tile_pool(name="mmp", bufs=2, space="PSUM") as mmp,
            tc.tile_pool(name="ptm", bufs=1, space="PSUM") as ptmp,
            tc.tile_pool(name="xnTp", bufs=1) as xnTp,
        ):
            xnT = xnTp.tile([P, DC, N], dt_mm)   # [dim%128, dimchunk, rows]
            vT = xnTp.tile([P, N], dt_mm)
            mustd_all = xnTp.tile([P, RT], dt_mm)
            diag_t = [xnTp.tile([P, 4, P], dt_mm, name=f"diag{g}")
                      for g in range(2)]

            # dependency-free burst to cover the first x DMA + cast + LN
            dummy_mms(20, width=RC_W)

            # b_out as a row (for the projection's rank-1 bias)
            bo_bf = xnTp.tile([P, 1], dt_mm)
            with nc.allow_low_precision(reason="bias cast"):
                nc.vector.tensor_copy(out=bo_bf, in_=bo_t)
            bp = ptmp.tile([4, P], f32, tag="ptm")
            nc.tensor.matmul(bp[0:1, 0:P], bo_bf[:, 0:1], ident,
                             start=True, stop=True)
            with nc.allow_low_precision(reason="bias row"):
                nc.vector.tensor_copy(out=bo_row[0:1, :], in_=bp[0:1, 0:P])

            for g4 in range(RT // 4):
                dg = diag_t[g4 % 2]
                for j in range(4):
                    rt = g4 * 4 + j
                    if rt < RT - 3:
                        nx = xp.tile([P, D], f32, tag="x", name=f"x{rt + 3}")
                        nc.sync.dma_start(
                            out=nx, in_=x_d[(rt + 3) * P:(rt + 4) * P, :]
                        )
                        x_tiles.append(nx)
                    x_t = x_tiles[rt]
                    xb = xbp.tile([P, D], dt_mm, tag="xb", name=f"xb{rt}")
                    nc.gpsimd.tensor_copy(out=xb, in_=x_t)
                    x_tiles[rt] = (x_t, xb)
                    stats = statp.tile([P, 2, 6], f32, tag="st")
                    for sg in range(2):
                        nc.vector.bn_stats(
                            out=stats[:, sg, :],
                            in_=xb[:, sg * 512:(sg + 1) * 512],
                        )
                    mv = statp.tile([P, 2], f32, tag="mv")
                    nc.vector.bn_aggr(out=mv, in_=stats)
                    rstd = statp.tile([P, 1], f32, tag="rstd")
                    nc.scalar.activation(
                        out=rstd, in_=mv[:, 1:2], func=AF.Sqrt,
                        bias=eps_t, scale=1.0,
                    )
                    nc.vector.reciprocal(out=rstd, in_=rstd)
                    with nc.allow_low_precision(reason="ln diag"):
                        nc.vector.tensor_scalar(
                            out=dg[:, j, :], in0=ident,
                            scalar1=rstd, scalar2=None,
                            op0=mybir.AluOpType.mult,
                        )
                    with nc.allow_low_precision(reason="mu*rstd"):
                        nc.vector.tensor_tensor(
                            out=mustd_all[:, rt:rt + 1], in0=mv[:, 0:1],
                            in1=rstd, op=mybir.AluOpType.mult,
                        )
                if g4 == 0:
                    nc.sync.dma_start(
                        out=wo_sb,
                        in_=wo_d[:, :].rearrange("(c p) m -> p c m", p=P),
                    )

                # transposes: xnT[d, r] = x[r, d] * rstd_r
                for j in range(4):
                    rt = g4 * 4 + j
                    xb = x_tiles[rt][1]
                    for g in range(2):
                        pt = tp.tile([P, 512], f32, tag="pt")
                        for jj in range(4):
                            dc = g * 4 + jj
                            nc.tensor.matmul(
                                pt[:, jj * P:(jj + 1) * P],
                                xb[:, dc * P:(dc + 1) * P],
                                dg[:, j, :],
                                start=True, stop=True,
                            )
                        dst = xnT[:, g * 4:(g + 1) * 4, rt * P:(rt + 1) * P]
                        tsrc = pt[:].rearrange("p (jj q) -> p jj q", jj=4)
                        with nc.allow_low_precision(reason="transpose evac"):
                            if (rt + g) % 2 == 0:
                                nc.vector.tensor_copy(out=dst, in_=tsrc)
                            else:
                                nc.scalar.copy(out=dst, in_=tsrc)
                    dummy_mms(1)

                # (mu*rstd) of this block's 4 tiles -> one row on partition 0
                nt = g4
                ptm = ptmp.tile([4, 512], f32, tag="ptm")
                for j in range(4):
                    rt = nt * 4 + j
                    nc.tensor.matmul(
                        ptm[0:1, j * P:(j + 1) * P],
                        mustd_all[:, rt:rt + 1],
                        ident, start=True, stop=True,
                    )
                with nc.allow_low_precision(reason="mustd row"):
                    nc.scalar.copy(
                        out=mustdT0[0:1, nt * 512:(nt + 1) * 512],
                        in_=ptm[0:1, :],
                    )

                # q/k/v projections for this 512-row block
                for w_sb, b_t, cs_t, dst in (
                    (wk_sb, kb_t, cs_k, kT), (wq_sb, qb_t, cs_q, qT),
                    (wv_sb, vb_t, cs_v, vT),
                ):
                    pm = mmp.tile([P, 512], f32, tag="pm")
                    for kc in range(DC):
                        nc.tensor.matmul(
                            pm,
                            w_sb[:, kc, :],
                            xnT[:, kc, nt * 512:(nt + 1) * 512],
                            start=(kc == 0), stop=False,
                        )
                    # rank-1 LN correction: -colsumW (x) mu*rstd
                    nc.tensor.matmul(
                        pm, cs_t,
                        mustdT0[0:1, nt * 512:(nt + 1) * 512],
                        start=False, stop=True,
                    )
                    nc.scalar.activation(
                        out=dst[:, nt * 512:(nt + 1) * 512], in_=pm,
                        func=AF.Identity, bias=b_t, scale=1.0,
                    )
                # v^T -> v (row-major with ones column) for this block
                for rt in range(g4 * 4, g4 * 4 + 4):
                    pt = tp.tile([P, 512], f32, tag="pt")
                    nc.tensor.matmul(
                        pt[:, :P], vT[:, rt * P:(rt + 1) * P], ident,
                        start=True, stop=True,
                    )
                    with nc.allow_low_precision(reason="v evac"):
                        nc.vector.tensor_copy(
                            out=v_sb[:, rt, :, 0:DH],
                            in_=pt[:, :P].rearrange("p (h d) -> p h d", h=HL),
                        )
                dummy_mms(1)

        # ---- stage D: attention -------------------------------------------
        with (
            tc.tile_pool(name="expp", bufs=2) as expp,
            tc.tile_pool(name="rsum", bufs=2) as rsump,
            tc.tile_pool(name="sp", bufs=2, space="PSUM") as sp,
            tc.tile_pool(name="op", bufs=1, space="PSUM") as op,
            tc.tile_pool(name="fp", bufs=1, space="PSUM") as fp,
            tc.tile_pool(name="agp", bufs=2) as agp,
        ):
            state = {}

            def sim_group(idx, kc):
                """Both heads' sim for one key chunk, row-group packed."""
                r0 = idx * RC_W
                st = state[idx]
                ps = sp.tile([P, 2 * RC_W], f32, tag="ps", name=f"ps{idx}_{kc}")
                for h in range(HL):
                    nc.tensor.matmul(
                        ps[:, h * RC_W:(h + 1) * RC_W],
                        kT[h * DH:(h + 1) * DH, kc * P:(kc + 1) * P],
                        qT[h * DH:(h + 1) * DH, r0:r0 + RC_W],
                        start=True, stop=True,
                    )
                nc.scalar.activation(
                    out=st["exp_t"][:, kc, :, :],
                    in_=ps[:].rearrange("p (c q) -> p c q", c=2),
                    func=AF.Exp, scale=SIM_SCALE,
                )

            def av_pair(idx, kc):
                """attn@v for key chunk kc, both heads."""
                st = state[idx]
                if st["po"] is None:
                    st["po"] = op.tile([P, 2 * RC_W], f32, tag="po",
                                       name=f"po{idx}")
                for h in range(HL):
                    nc.tensor.matmul(
                        st["po"][0:DH + 1, h * RC_W:(h + 1) * RC_W],
                        v_sb[:, kc, h, :],
                        st["exp_t"][:, kc, h, :],
                        start=(kc == 0), stop=(kc == RT - 1),
                    )

            def den_prep(idx):
                """Denominators PSUM -> SBUF (single lane, off-PE)."""
                st = state[idx]
                den = rsump.tile([DH + 1, 2 * RC_W], dt_mm, tag="den",
                                 name=f"den{idx}")
                st["den"] = den
                with nc.allow_low_precision(reason="softmax denom"):
                    nc.vector.tensor_copy(
                        out=den[DH:DH + 1, :], in_=st["po"][DH:DH + 1, :]
                    )

            def den_bcast(idx):
                """Broadcast denominators to 64 lanes (PE rank-1), recip."""
                st = state[idx]
                den = st["den"]
                pr = sp.tile([P, 2 * RC_W], f32, tag="ps", name=f"pr{idx}")
                for h in range(HL):
                    nc.tensor.matmul(
                        pr[0:DH, h * RC_W:(h + 1) * RC_W],
                        ones_row[DH:DH + 1, 0:DH],
                        den[DH:DH + 1, h * RC_W:(h + 1) * RC_W],
                        start=True, stop=True,
                    )
                rb = rsump.tile([DH, 2 * RC_W], f32, tag="rb", name=f"rb{idx}")
                st["rb"] = rb
                nc.vector.reciprocal(out=rb, in_=pr[0:DH, :])

            def norm_fin(idx):
                """Normalize, ship both heads, AllGather."""
                st = state[idx]
                r0 = idx * RC_W
                for h in range(HL):
                    with nc.allow_low_precision(reason="attn bf16 wire"):
                        nc.vector.tensor_tensor(
                            out=attn_h[h][:, r0:r0 + RC_W],
                            in0=st["po"][0:DH, h * RC_W:(h + 1) * RC_W],
                            in1=st["rb"][:, h * RC_W:(h + 1) * RC_W],
                            op=mybir.AluOpType.mult,
                        )
                    nc.sync.dma_start(
                        out=ag_in[idx][h * DH:(h + 1) * DH, :],
                        in_=attn_h[h][:, r0:r0 + RC_W],
                    )
                nc.gpsimd.collective_compute(
                    "AllGather",
                    mybir.AluOpType.bypass,
                    replica_groups=groups,
                    ins=[ag_in[idx][:].opt()],
                    outs=[ag_out[idx][:].opt()],
                )

            def proj_dma(idx):
                """Emit the gathered-heads loads for row chunk idx."""
                st = state[idx]
                agt = agp.tile([P, DC, RC_W], dt_mm, tag="agt",
                               name=f"agt{idx}")
                st["agt"] = agt
                for kc in range(DC):
                    nc.sync.dma_start(
                        out=agt[:, kc, :],
                        in_=ag_out[idx][kc * P:(kc + 1) * P, :],
                    )

            def proj(idx):
                """outT slice for row chunk idx (+ rank-1 bias), to DRAM."""
                st = state[idx]
                r0 = idx * RC_W
                agt = st["agt"]
                pf = fp.tile([P, RC_W], f32, tag="pf", name=f"pf{idx}")
                for kc in range(DC):
                    nc.tensor.matmul(
                        pf, wo_sb[:, kc, :], agt[:, kc, :],
                        start=(kc == 0), stop=False,
                    )
                nc.tensor.matmul(
                    pf, bo_row[0:1, :], ones_row[0:1, :],
                    start=False, stop=True,
                )
                ot = agp.tile([P, RC_W], f32, tag="ot", name=f"ot{idx}")
                nc.vector.tensor_copy(out=ot, in_=pf)
                nc.sync.dma_start(out=out_d[:, r0:r0 + RC_W], in_=ot)

            for idx in range(NRC):
                state[idx] = {
                    "exp_t": expp.tile([P, RT, HL, RC_W], dt_mm, tag="exp",
                                       name=f"exp{idx}"),
                    "po": None, "den": None, "rb": None, "agt": None,
                }
                if idx > 0:
                    den_bcast(idx - 1)
                for kc in range(RT):
                    sim_group(idx, kc)
                    if kc >= AV_LAG:
                        av_pair(idx, kc - AV_LAG)
                    dummy_mms(1)
                    if idx > 0:
                        if kc == 2:
                            norm_fin(idx - 1)
                        elif kc == 8:
                            proj_dma(idx - 1)
                        elif kc == 14:
                            proj(idx - 1)
                for kc in range(RT - AV_LAG, RT):
                    av_pair(idx, kc)
                den_prep(idx)
            # tail: last stage's normalize + gather + projection
            den_bcast(NRC - 1)
            norm_fin(NRC - 1)
            proj_dma(NRC - 1)
            proj(NRC - 1)

            if DEBUG:
                dbg_sb = agp.tile([P, N + 128], f32, tag="dbg")
                for src, dst in ((qT, dbg_qt), (kT, dbg_kt)):
                    nc.vector.tensor_copy(out=dbg_sb[:, 0:N], in_=src)
                    nc.sync.dma_start(out=dst[:, :], in_=dbg_sb[:, 0:N])
                nc.vector.tensor_copy(
                    out=dbg_sb[:, 0:RT * HL * (DH + 1)],
                    in_=v_sb[:].rearrange("p a b c -> p (a b c)"),
                )
                nc.sync.dma_start(out=dbg_v[:, :],
                                  in_=dbg_sb[:, 0:RT * HL * (DH + 1)])
                nc.vector.tensor_copy(out=dbg_sb[0:DH + 1, 0:2 * RC_W],
                                      in_=state[0]["den"])
                nc.sync.dma_start(out=dbg_den[:, :],
                                  in_=dbg_sb[0:DH + 1, 0:2 * RC_W])
                nc.vector.tensor_copy(out=dbg_sb[0:DH, 0:N], in_=attn_h[0])
                nc.sync.dma_start(out=dbg_attn[:, :], in_=dbg_sb[0:DH, 0:N])

    if not nc.is_finalized():
        nc.finalize()
    return nc


def _get_built():
    global _BUILT
    if _BUILT is None:
        _BUILT = _build()
    return _BUILT


def _shard_inputs(x, ln_scale, ln_bias, w_qkv, w_out, b_out):
    """Host-side sharding: slice per-head weight columns, fold LN params."""
    x = np.ascontiguousarray(np.asarray(x, np.float32))
    ln_scale = np.asarray(ln_scale, np.float32)
    ln_bias = np.asarray(ln_bias, np.float32)
    w_qkv = np.asarray(w_qkv, np.float32)
    w_out = np.asarray(w_out, np.float32)
    b_out = np.asarray(b_out, np.float32)

    w_np = {"f32": np.float32, "f32r": np.float32,
            "bf16": ml_dtypes.bfloat16}[MM_DT]

    in_maps = []
    for ci in range(NCORES):
        c0 = ci * HC
        sl = {}
        for name, off in (("q", 0), ("k", HEADS * DH), ("v", 2 * HEADS * DH)):
            w = w_qkv[:, off + c0: off + c0 + HC]
            wf = ln_scale[:, None] * w
            sl["w" + name] = np.ascontiguousarray(wf.astype(w_np))
            sl[name + "b"] = np.ascontiguousarray(ln_bias @ w)
            sl[name + "cs"] = np.ascontiguousarray(
                (-wf.sum(axis=0)).astype(w_np)
            )
        sl["wo"] = np.ascontiguousarray(w_out[:, c0:c0 + HC].astype(w_np))
        sl["bo"] = np.ascontiguousarray(b_out[c0:c0 + HC])
        sl["x"] = x
        in_maps.append(sl)
    return in_maps


def kernel(x, ln_scale, ln_bias, w_qkv, w_out, b_out):
    from concourse.bass_utils import run_bass_kernel_spmd

    nc = _get_built()
    in_maps = _shard_inputs(x, ln_scale, ln_bias, w_qkv, w_out, b_out)
    res = run_bass_kernel_spmd(nc, in_maps, core_ids=list(range(NCORES)))
    shards = [res.results[ci]["out"] for ci in range(NCORES)]  # [128, 2048] each
    outT = np.concatenate(shards, axis=0)  # [1024, 2048]
    return np.ascontiguousarray(outT.T)


# revision 28
# speedup vs baseline: 1.0160x; 1.0160x over previous
"""Distributed Trainium2 kernel for pre-LN multi-head self-attention.

Reference computation (n=2048, d=1024, 16 heads x 64):
    xn  = LayerNorm(x) * ln_scale + ln_bias
    qkv = xn @ w_qkv ; split -> q,k,v [16, 2048, 64]
    sim = (q @ k^T) * d**-0.5 ; attn = softmax(sim)
    out = concat_heads(attn @ v) @ w_out + b_out

Sharding: 2 heads per core (tensor parallel). Each core:
  - computes LayerNorm(x) (replicated) and xn^T via PE transposes
    (ln_scale folded into weights on host, ln_bias folded into a
    per-output-column bias added at PSUM evacuation)
  - projects its 2 heads' q/k/v; attention in transposed layout (keys
    on partitions); a ones column in v yields softmax denominators free
  - denominators: PE rank-1 broadcast then per-head 64-lane reciprocals
  - one AllGather per row-chunk stage (both heads packed [128, w]);
    stage widths taper (512,512,512,384,128) so the tail AllGather that
    cannot be overlapped is small
  - computes a 128-column slice of the final projection (+ rank-1 bias)
Host assembles the 8 [128, 2048] outT shards into [2048, 1024].

Scheduling: one continuous global key-chunk stream — sim(s,kc) slots
run back to back across stage boundaries with attn@v trailing at a
fixed global lag, so neither PE nor ACT ever hits a stage-boundary
barrier. Norm/AllGather chains and projections are slotted into the
stream at fixed offsets. Dummy matmuls cover the prologue's DMA waits
to keep the HAM clock gate warm.
"""

import sys

import ml_dtypes
import numpy as np

for _p in ("/opt/trn_rl_repo", "/root/.axon_site/_ro/trn_rl_repo"):
    if _p not in sys.path:
        sys.path.append(_p)

N = 2048          # sequence length
D = 1024          # model dim
HEADS = 16
DH = 64
NCORES = 8
HL = HEADS // NCORES          # heads per core (2)
HC = HL * DH                  # head cols per core (128)
LN_EPS = 1e-6
SIM_SCALE = float(D) ** -0.5  # reference scales by input dim

P = 128
RT = N // P        # 16 row tiles
DC = D // P        # 8 dim chunks
RC_W = 512         # max row-chunk width
CHUNKS = [(0, 512), (512, 512), (1024, 512), (1536, 384), (1920, 128)]
S = len(CHUNKS)
LAG = 5            # attn@v trails sim by this many global slots

MM_DT = "bf16"
DEBUG = False

_BUILT = None


def _build():
    """Build the SPMD Bass graph (same graph on all 8 cores)."""
    from contextlib import ExitStack

    import concourse.tile as tile
    from concourse import bacc, mybir
    from concourse.masks import make_identity

    f32 = mybir.dt.float32
    dt_mm = {"f32": f32, "f32r": mybir.dt.float32r,
             "bf16": mybir.dt.bfloat16}[MM_DT]
    AF = mybir.ActivationFunctionType

    nc = bacc.Bacc(None, num_devices=NCORES)

    x_d = nc.declare_dram_parameter("x", [N, D], f32, isOutput=False)
    wq_d = nc.declare_dram_parameter("wq", [D, HC], dt_mm, isOutput=False)
    wk_d = nc.declare_dram_parameter("wk", [D, HC], dt_mm, isOutput=False)
    wv_d = nc.declare_dram_parameter("wv", [D, HC], dt_mm, isOutput=False)
    qb_d = nc.declare_dram_parameter("qb", [HC], f32, isOutput=False)
    kb_d = nc.declare_dram_parameter("kb", [HC], f32, isOutput=False)
    vb_d = nc.declare_dram_parameter("vb", [HC], f32, isOutput=False)
    wo_d = nc.declare_dram_parameter("wo", [D, HC], dt_mm, isOutput=False)
    bo_d = nc.declare_dram_parameter("bo", [HC], f32, isOutput=False)
    out_d = nc.declare_dram_parameter("out", [HC, N], f32, isOutput=True)
    if DEBUG:
        dbg_qt = nc.declare_dram_parameter("dbg_qt", [P, N], f32, isOutput=True)
        dbg_kt = nc.declare_dram_parameter("dbg_kt", [P, N], f32, isOutput=True)
        dbg_v = nc.declare_dram_parameter("dbg_v", [P, RT * HL * (DH + 1)],
                                          f32, isOutput=True)
        dbg_den = nc.declare_dram_parameter("dbg_den", [DH + 1, 2 * RC_W],
                                            f32, isOutput=True)
        dbg_attn = nc.declare_dram_parameter("dbg_attn", [DH, N], f32,
                                             isOutput=True)

    groups = [list(range(NCORES))]

    with ExitStack() as ctx:
        tc = ctx.enter_context(tile.TileContext(nc))

        dram = ctx.enter_context(tc.tile_pool(name="dram", bufs=1, space="DRAM"))
        ag_in = [dram.tile([P, w], dt_mm, name=f"ag_in{i}")
                 for i, (_, w) in enumerate(CHUNKS)]
        ag_out = [dram.tile([NCORES * P, w], dt_mm, addr_space="Shared",
                            name=f"ag_out{i}") for i, (_, w) in enumerate(CHUNKS)]

        singles = ctx.enter_context(tc.tile_pool(name="singles", bufs=1))
        xp = ctx.enter_context(tc.tile_pool(name="xp", bufs=6))
        xhp = ctx.enter_context(tc.tile_pool(name="xh", bufs=6))

        ident = singles.tile([P, P], dt_mm)
        make_identity(nc, ident)
        warm_rhs = singles.tile([P, RC_W], dt_mm)
        nc.vector.memset(warm_rhs, 0.0)
        eps_t = singles.tile([P, 1], f32)
        nc.vector.memset(eps_t, LN_EPS)
        ones_row = singles.tile([P, RC_W], dt_mm)
        nc.gpsimd.memset(ones_row, 1.0)

        # x tile DMAs first so tile 0's pipeline starts ASAP
        x_tiles = []
        for rt in range(3):
            x_t = xp.tile([P, D], f32, tag="x", name=f"x{rt}")
            nc.sync.dma_start(out=x_t, in_=x_d[rt * P:(rt + 1) * P, :])
            x_tiles.append(x_t)

        # weights / biases (wk first: K projection runs first)
        wq_sb = singles.tile([P, DC, HC], dt_mm)
        wk_sb = singles.tile([P, DC, HC], dt_mm)
        wv_sb = singles.tile([P, DC, HC], dt_mm)
        wo_sb = singles.tile([P, DC, HC], dt_mm)
        nc.sync.dma_start(
            out=wk_sb, in_=wk_d[:, :].rearrange("(c p) m -> p c m", p=P)
        )
        nc.sync.dma_start(
            out=wq_sb, in_=wq_d[:, :].rearrange("(c p) m -> p c m", p=P)
        )
        nc.sync.dma_start(
            out=wv_sb, in_=wv_d[:, :].rearrange("(c p) m -> p c m", p=P)
        )
        qb_t = singles.tile([P, 1], f32)
        kb_t = singles.tile([P, 1], f32)
        vb_t = singles.tile([P, 1], f32)
        bo_t = singles.tile([P, 1], f32)
        for b_t, b_d in ((qb_t, qb_d), (kb_t, kb_d), (vb_t, vb_d), (bo_t, bo_d)):
            nc.sync.dma_start(out=b_t, in_=b_d[:].rearrange("(p o) -> p o", o=1))

        # long-lived activations
        qT = singles.tile([P, N], dt_mm)        # [2*64 qdims, rows]
        kT = singles.tile([P, N], dt_mm)
        v_sb = singles.tile([P, RT, HL, DH + 1], dt_mm)  # [key%128, kc, h, v|1]
        attn_h = [singles.tile([DH, N], dt_mm, name=f"attn_h{h}")
                  for h in range(HL)]
        bo_row = singles.tile([4, P], dt_mm)     # row 0 = b_out slice as a row

        nc.gpsimd.memset(v_sb[:, :, :, DH:], 1.0)  # ones column

        # ---- stages A-C: LayerNorm -> xn^T -> q/k/v ------------------------
        with (
            tc.tile_pool(name="stat", bufs=4) as statp,
            tc.tile_pool(name="tp", bufs=2, space="PSUM") as tp,
            tc.tile_pool(name="mmp", bufs=2, space="PSUM") as mmp,
            tc.tile_pool(name="ptm", bufs=1, space="PSUM") as ptmp,
            tc.tile_pool(name="dum", bufs=1, space="PSUM") as dump,
            tc.tile_pool(name="xnTp", bufs=1) as xnTp,
        ):
            xnT = xnTp.tile([P, DC, N], dt_mm)   # [dim%128, dimchunk, rows]
            vT = xnTp.tile([P, N], dt_mm)

            dum_ps = dump.tile([P, RC_W], f32, tag="warm")

            def dummy_mms(n, width=RC_W):
                for _ in range(n):
                    nc.tensor.matmul(dum_ps[:, 0:width], ident,
                                     warm_rhs[:, 0:width], start=True,
                                     stop=True)

            # dependency-free burst to cover the first x DMA + LN latency
            dummy_mms(22)

            # b_out as a row (for the projection's rank-1 bias)
            bo_bf = xnTp.tile([P, 1], dt_mm)
            with nc.allow_low_precision(reason="bias cast"):
                nc.vector.tensor_copy(out=bo_bf, in_=bo_t)
            bp = ptmp.tile([4, P], f32, tag="ptm")
            nc.tensor.matmul(bp[0:1, 0:P], bo_bf[:, 0:1], ident,
                             start=True, stop=True)
            with nc.allow_low_precision(reason="bias row"):
                nc.vector.tensor_copy(out=bo_row[0:1, :], in_=bp[0:1, 0:P])

            for g4 in range(RT // 4):
                rstd4 = statp.tile([P, 4], f32, tag="rstd")
                for j in range(4):
                    rt = g4 * 4 + j
                    if rt < RT - 3:
                        nx = xp.tile([P, D], f32, tag="x", name=f"x{rt + 3}")
                        nc.sync.dma_start(
                            out=nx, in_=x_d[(rt + 3) * P:(rt + 4) * P, :]
                        )
                        x_tiles.append(nx)
                    x_t = x_tiles[rt]
                    stats = statp.tile([P, 2, 6], f32, tag="st")
                    for sg in range(2):
                        nc.vector.bn_stats(
                            out=stats[:, sg, :],
                            in_=x_t[:, sg * 512:(sg + 1) * 512],
                        )
                    mv = statp.tile([P, 2], f32, tag=f"mv{j}")
                    nc.vector.bn_aggr(out=mv, in_=stats)
                    nc.scalar.activation(
                        out=rstd4[:, j:j + 1], in_=mv[:, 1:2], func=AF.Sqrt,
                        bias=eps_t, scale=1.0,
                    )
                    x_tiles[rt] = (x_t, mv)
                nc.vector.reciprocal(out=rstd4, in_=rstd4)
                if g4 == 0:
                    nc.sync.dma_start(
                        out=wo_sb,
                        in_=wo_d[:, :].rearrange("(c p) m -> p c m", p=P),
                    )

                for j in range(4):
                    rt = g4 * 4 + j
                    x_t, mv = x_tiles[rt]
                    xh_t = xhp.tile([P, D], dt_mm, tag="xh")
                    nc.vector.tensor_scalar(
                        out=xh_t, in0=x_t,
                        scalar1=mv[:, 0:1], scalar2=rstd4[:, j:j + 1],
                        op0=mybir.AluOpType.subtract,
                        op1=mybir.AluOpType.mult,
                    )
                    for g in range(2):
                        pt = tp.tile([P, 512], f32, tag="pt")
                        for jj in range(4):
                            dc = g * 4 + jj
                            nc.tensor.matmul(
                                pt[:, jj * P:(jj + 1) * P],
                                xh_t[:, dc * P:(dc + 1) * P],
                                ident, start=True, stop=True,
                            )
                        dst = xnT[:, g * 4:(g + 1) * 4, rt * P:(rt + 1) * P]
                        tsrc = pt[:].rearrange("p (jj q) -> p jj q", jj=4)
                        with nc.allow_low_precision(reason="transpose evac"):
                            if (rt + 2 * g) % 4 == 0:
                                nc.vector.tensor_copy(out=dst, in_=tsrc)
                            else:
                                nc.scalar.copy(out=dst, in_=tsrc)
                    dummy_mms(1)

                # q/k/v projections for this 512-row block
                nt = g4
                for w_sb, b_t, dst in (
                    (wk_sb, kb_t, kT), (wq_sb, qb_t, qT), (wv_sb, vb_t, vT),
                ):
                    pm = mmp.tile([P, 512], f32, tag="pm")
                    for kc in range(DC):
                        nc.tensor.matmul(
                            pm,
                            w_sb[:, kc, :],
                            xnT[:, kc, nt * 512:(nt + 1) * 512],
                            start=(kc == 0), stop=(kc == DC - 1),
                        )
                    nc.scalar.activation(
                        out=dst[:, nt * 512:(nt + 1) * 512], in_=pm,
                        func=AF.Identity, bias=b_t, scale=1.0,
                    )
                # v^T -> v (row-major with ones column) for this block
                for rt in range(g4 * 4, g4 * 4 + 4):
                    pt = tp.tile([P, 512], f32, tag="pt")
                    nc.tensor.matmul(
                        pt[:, :P], vT[:, rt * P:(rt + 1) * P], ident,
                        start=True, stop=True,
                    )
                    with nc.allow_low_precision(reason="v evac"):
                        nc.vector.tensor_copy(
                            out=v_sb[:, rt, :, 0:DH],
                            in_=pt[:, :P].rearrange("p (h d) -> p h d", h=HL),
                        )
                dummy_mms(1)

        # ---- stage D: attention, one continuous global kc stream -----------
        with (
            tc.tile_pool(name="expp", bufs=2) as expp,
            tc.tile_pool(name="rsum", bufs=2) as rsump,
            tc.tile_pool(name="sp", bufs=2, space="PSUM") as sp,
            tc.tile_pool(name="op", bufs=2, space="PSUM") as op,
            tc.tile_pool(name="agp", bufs=2) as agp,
        ):
            state = {}

            def sim_group(s, kc):
                r0, w = CHUNKS[s]
                st = state[s]
                ps = sp.tile([P, 2 * RC_W], f32, tag="ps", name=f"ps{s}_{kc}")
                for h in range(HL):
                    nc.tensor.matmul(
                        ps[:, h * RC_W:h * RC_W + w],
                        kT[h * DH:(h + 1) * DH, kc * P:(kc + 1) * P],
                        qT[h * DH:(h + 1) * DH, r0:r0 + w],
                        start=True, stop=True,
                    )
                nc.scalar.activation(
                    out=st["exp_t"][:, kc, :, 0:w],
                    in_=ps[:].rearrange("p (c q) -> p c q", c=2)[:, :, 0:w],
                    func=AF.Exp, scale=SIM_SCALE,
                )

            def av_pair(s, kc):
                _, w = CHUNKS[s]
                st = state[s]
                if st["po"] is None:
                    st["po"] = op.tile([P, 2 * RC_W], f32, tag="po",
                                       name=f"po{s}")
                for h in range(HL):
                    nc.tensor.matmul(
                        st["po"][0:DH + 1, h * RC_W:h * RC_W + w],
                        v_sb[:, kc, h, :],
                        st["exp_t"][:, kc, h, 0:w],
                        start=(kc == 0), stop=(kc == RT - 1),
                    )

            def norm_chain(s):
                """Denominators -> reciprocals -> normalize -> AllGather."""
                r0, w = CHUNKS[s]
                st = state[s]
                den = rsump.tile([DH + 1, 2 * RC_W], dt_mm, tag="den",
                                 name=f"den{s}")
                with nc.allow_low_precision(reason="softmax denom"):
                    nc.vector.tensor_copy(
                        out=den[DH:DH + 1, :], in_=st["po"][DH:DH + 1, :]
                    )
                pr = sp.tile([P, 2 * RC_W], f32, tag="ps", name=f"pr{s}")
                for h in range(HL):
                    nc.tensor.matmul(
                        pr[0:DH, h * RC_W:h * RC_W + w],
                        ones_row[DH:DH + 1, 0:DH],
                        den[DH:DH + 1, h * RC_W:h * RC_W + w],
                        start=True, stop=True,
                    )
                rb = rsump.tile([DH, 2 * RC_W], f32, tag="rb", name=f"rb{s}")
                for h in range(HL):
                    nc.vector.reciprocal(
                        out=rb[:, h * RC_W:h * RC_W + w],
                        in_=pr[0:DH, h * RC_W:h * RC_W + w],
                    )
                    with nc.allow_low_precision(reason="attn bf16 wire"):
                        nc.vector.tensor_tensor(
                            out=attn_h[h][:, r0:r0 + w],
                            in0=st["po"][0:DH, h * RC_W:h * RC_W + w],
                            in1=rb[:, h * RC_W:h * RC_W + w],
                            op=mybir.AluOpType.mult,
                        )
                    nc.sync.dma_start(
                        out=ag_in[s][h * DH:(h + 1) * DH, :],
                        in_=attn_h[h][:, r0:r0 + w],
                    )
                nc.gpsimd.collective_compute(
                    "AllGather",
                    mybir.AluOpType.bypass,
                    replica_groups=groups,
                    ins=[ag_in[s][:].opt()],
                    outs=[ag_out[s][:].opt()],
                )
                if DEBUG:
                    st["den_t"] = den

            def proj_dma(s):
                _, w = CHUNKS[s]
                st = state[s]
                agt = agp.tile([P, DC, RC_W], dt_mm, tag="agt", name=f"agt{s}")
                st["agt"] = agt
                for kc in range(DC):
                    nc.sync.dma_start(
                        out=agt[:, kc, 0:w],
                        in_=ag_out[s][kc * P:(kc + 1) * P, :],
                    )

            def proj(s):
                r0, w = CHUNKS[s]
                st = state[s]
                agt = st["agt"]
                pf = sp.tile([P, 2 * RC_W], f32, tag="ps", name=f"pf{s}")
                for kc in range(DC):
                    nc.tensor.matmul(
                        pf[:, 0:w], wo_sb[:, kc, :], agt[:, kc, 0:w],
                        start=(kc == 0), stop=False,
                    )
                nc.tensor.matmul(
                    pf[:, 0:w], bo_row[0:1, :], ones_row[0:1, 0:w],
                    start=False, stop=True,
                )
                ot = agp.tile([P, RC_W], f32, tag="ot", name=f"ot{s}")
                nc.vector.tensor_copy(out=ot[:, 0:w], in_=pf[:, 0:w])
                nc.sync.dma_start(out=out_d[:, r0:r0 + w], in_=ot[:, 0:w])

            for s in range(S):
                state[s] = {
                    "exp_t": expp.tile([P, RT, HL, RC_W], dt_mm, tag="exp",
                                       name=f"exp{s}"),
                    "po": None, "agt": None,
                }

            total = S * RT + LAG
            for t in range(total):
                s, kc = divmod(t, RT)
                if t < S * RT:
                    sim_group(s, kc)
                u = t - LAG
                if u >= 0:
                    us, ukc = divmod(u, RT)
                    av_pair(us, ukc)
                    if ukc == RT - 1:
                        norm_chain(us)
                if t % RT == 8 and 2 <= t // RT <= S - 1:
                    proj_dma(t // RT - 2)
                if t % RT == 13 and 2 <= t // RT <= S - 1:
                    proj(t // RT - 2)
            # drain: remaining projections
            proj_dma(S - 2)
            proj(S - 2)
            proj_dma(S - 1)
            proj(S - 1)

            if DEBUG:
                dbg_sb = agp.tile([P, N + 128], f32, tag="dbg")
                for src, dst in ((qT, dbg_qt), (kT, dbg_kt)):
                    nc.vector.tensor_copy(out=dbg_sb[:, 0:N], in_=src)
                    nc.sync.dma_start(out=dst[:, :], in_=dbg_sb[:, 0:N])
                nc.vector.tensor_copy(
                    out=dbg_sb[:, 0:RT * HL * (DH + 1)],
                    in_=v_sb[:].rearrange("p a b c -> p (a b c)"),
                )
                nc.sync.dma_start(out=dbg_v[:, :],
                                  in_=dbg_sb[:, 0:RT * HL * (DH + 1)])
                nc.vector.tensor_copy(out=dbg_sb[0:DH + 1, 0:2 * RC_W],
                                      in_=state[0]["den_t"])
                nc.sync.dma_start(out=dbg_den[:, :],
                                  in_=dbg_sb[0:DH + 1, 0:2 * RC_W])
                nc.vector.tensor_copy(out=dbg_sb[0:DH, 0:N], in_=attn_h[0])
                nc.sync.dma_start(out=dbg_attn[:, :], in_=dbg_sb[0:DH, 0:N])

    if not nc.is_finalized():
        nc.finalize()
    return nc


def _get_built():
    global _BUILT
    if _BUILT is None:
        _BUILT = _build()
    return _BUILT


def _shard_inputs(x, ln_scale, ln_bias, w_qkv, w_out, b_out):
    """Host-side sharding: slice per-head weight columns, fold LN params."""
    x = np.ascontiguousarray(np.asarray(x, np.float32))
    ln_scale = np.asarray(ln_scale, np.float32)
    ln_bias = np.asarray(ln_bias, np.float32)
    w_qkv = np.asarray(w_qkv, np.float32)
    w_out = np.asarray(w_out, np.float32)
    b_out = np.asarray(b_out, np.float32)

    w_np = {"f32": np.float32, "f32r": np.float32,
            "bf16": ml_dtypes.bfloat16}[MM_DT]

    in_maps = []
    for ci in range(NCORES):
        c0 = ci * HC
        sl = {}
        for name, off in (("q", 0), ("k", HEADS * DH), ("v", 2 * HEADS * DH)):
            w = w_qkv[:, off + c0: off + c0 + HC]
            sl["w" + name] = np.ascontiguousarray(
                (ln_scale[:, None] * w).astype(w_np)
            )
            sl[name + "b"] = np.ascontiguousarray(ln_bias @ w)
        sl["wo"] = np.ascontiguousarray(w_out[:, c0:c0 + HC].astype(w_np))
        sl["bo"] = np.ascontiguousarray(b_out[c0:c0 + HC])
        sl["x"] = x
        in_maps.append(sl)
    return in_maps


def kernel(x, ln_scale, ln_bias, w_qkv, w_out, b_out):
    from concourse.bass_utils import run_bass_kernel_spmd

    nc = _get_built()
    in_maps = _shard_inputs(x, ln_scale, ln_bias, w_qkv, w_out, b_out)
    res = run_bass_kernel_spmd(nc, in_maps, core_ids=list(range(NCORES)))
    shards = [res.results[ci]["out"] for ci in range(NCORES)]  # [128, 2048] each
    outT = np.concatenate(shards, axis=0)  # [1024, 2048]
    return np.ascontiguousarray(outT.T)


# revision 40
# speedup vs baseline: 1.0283x; 1.0121x over previous
"""Distributed Trainium2 kernel for pre-LN multi-head self-attention.

Reference computation (n=2048, d=1024, 16 heads x 64):
    xn  = LayerNorm(x) * ln_scale + ln_bias
    qkv = xn @ w_qkv ; split -> q,k,v [16, 2048, 64]
    sim = (q @ k^T) * d**-0.5 ; attn = softmax(sim)
    out = concat_heads(attn @ v) @ w_out + b_out

Sharding: 2 heads per core (tensor parallel). Each core:
  - computes LayerNorm(x) (replicated) and xn^T via PE transposes
    (ln_scale folded into weights on host, ln_bias folded into a
    per-output-column bias added at PSUM evacuation)
  - projects its 2 heads' q/k/v; attention in transposed layout (keys
    on partitions); a ones column in v yields softmax denominators free
  - denominators: PE rank-1 broadcast then per-head 64-lane reciprocals
  - one AllGather per row-chunk stage (both heads packed [128, w]);
    stage widths taper (512,512,512,384,128) so the tail AllGather that
    cannot be overlapped is small
  - computes a 128-column slice of the final projection (+ rank-1 bias)
Host assembles the 8 [128, 2048] outT shards into [2048, 1024].

Scheduling: one continuous global key-chunk stream — sim(s,kc) slots
run back to back across stage boundaries with attn@v trailing at a
fixed global lag, so neither PE nor ACT ever hits a stage-boundary
barrier. Norm/AllGather chains and projections are slotted into the
stream at fixed offsets. Dummy matmuls cover the prologue's DMA waits
to keep the HAM clock gate warm.
"""

import sys

import ml_dtypes
import numpy as np

for _p in ("/opt/trn_rl_repo", "/root/.axon_site/_ro/trn_rl_repo"):
    if _p not in sys.path:
        sys.path.append(_p)

N = 2048          # sequence length
D = 1024          # model dim
HEADS = 16
DH = 64
NCORES = 8
HL = HEADS // NCORES          # heads per core (2)
HC = HL * DH                  # head cols per core (128)
LN_EPS = 1e-6
SIM_SCALE = float(D) ** -0.5  # reference scales by input dim

P = 128
RT = N // P        # 16 row tiles
DC = D // P        # 8 dim chunks
RC_W = 512         # max row-chunk width
CHUNKS = [(0, 512), (512, 512), (1024, 512), (1536, 384), (1920, 128)]
S = len(CHUNKS)
LAG = 10           # attn@v trails sim by this many global slots

MM_DT = "bf16"
DEBUG = False

_BUILT = None


def _build():
    """Build the SPMD Bass graph (same graph on all 8 cores)."""
    from contextlib import ExitStack

    import concourse.tile as tile
    from concourse import bacc, mybir
    from concourse.masks import make_identity

    f32 = mybir.dt.float32
    dt_mm = {"f32": f32, "f32r": mybir.dt.float32r,
             "bf16": mybir.dt.bfloat16}[MM_DT]
    AF = mybir.ActivationFunctionType

    nc = bacc.Bacc(None, num_devices=NCORES)

    x_d = nc.declare_dram_parameter("x", [N, D], f32, isOutput=False)
    wq_d = nc.declare_dram_parameter("wq", [D, HC], dt_mm, isOutput=False)
    wk_d = nc.declare_dram_parameter("wk", [D, HC], dt_mm, isOutput=False)
    wv_d = nc.declare_dram_parameter("wv", [D, HC], dt_mm, isOutput=False)
    qb_d = nc.declare_dram_parameter("qb", [HC], f32, isOutput=False)
    kb_d = nc.declare_dram_parameter("kb", [HC], f32, isOutput=False)
    vb_d = nc.declare_dram_parameter("vb", [HC], f32, isOutput=False)
    wo_d = nc.declare_dram_parameter("wo", [D, HC], dt_mm, isOutput=False)
    bo_d = nc.declare_dram_parameter("bo", [HC], f32, isOutput=False)
    out_d = nc.declare_dram_parameter("out", [HC, N], f32, isOutput=True)
    if DEBUG:
        dbg_qt = nc.declare_dram_parameter("dbg_qt", [P, N], f32, isOutput=True)
        dbg_kt = nc.declare_dram_parameter("dbg_kt", [P, N], f32, isOutput=True)
        dbg_v = nc.declare_dram_parameter("dbg_v", [P, RT * HL * (DH + 1)],
                                          f32, isOutput=True)
        dbg_den = nc.declare_dram_parameter("dbg_den", [DH + 1, 2 * RC_W],
                                            f32, isOutput=True)
        dbg_attn = nc.declare_dram_parameter("dbg_attn", [DH, N], f32,
                                             isOutput=True)

    groups = [list(range(NCORES))]

    with ExitStack() as ctx:
        tc = ctx.enter_context(tile.TileContext(nc))

        dram = ctx.enter_context(tc.tile_pool(name="dram", bufs=1, space="DRAM"))
        ag_in = [dram.tile([P, w], dt_mm, name=f"ag_in{i}")
                 for i, (_, w) in enumerate(CHUNKS)]
        ag_out = [dram.tile([NCORES * P, w], dt_mm, addr_space="Shared",
                            name=f"ag_out{i}") for i, (_, w) in enumerate(CHUNKS)]

        singles = ctx.enter_context(tc.tile_pool(name="singles", bufs=1))
        xp = ctx.enter_context(tc.tile_pool(name="xp", bufs=10))
        xhp = ctx.enter_context(tc.tile_pool(name="xh", bufs=8))
        dump = ctx.enter_context(tc.tile_pool(name="dum", bufs=1, space="PSUM"))

        ident = singles.tile([P, P], dt_mm)
        make_identity(nc, ident)
        warm_rhs = singles.tile([P, RC_W], dt_mm)
        nc.vector.memset(warm_rhs, 0.0)
        eps_t = singles.tile([P, 1], f32)
        nc.vector.memset(eps_t, LN_EPS)
        ones_row = singles.tile([P, RC_W], dt_mm)
        nc.gpsimd.memset(ones_row, 1.0)

        # x tile DMAs first so tile 0's pipeline starts ASAP
        x_tiles = []
        for rt in range(3):
            x_t = xp.tile([P, D], f32, tag="x", name=f"x{rt}")
            nc.sync.dma_start(out=x_t, in_=x_d[rt * P:(rt + 1) * P, :])
            x_tiles.append(x_t)

        # weights / biases (wk first: K projection runs first)
        wq_sb = singles.tile([P, DC, HC], dt_mm)
        wk_sb = singles.tile([P, DC, HC], dt_mm)
        wv_sb = singles.tile([P, DC, HC], dt_mm)
        wo_sb = singles.tile([P, DC, HC], dt_mm)
        nc.sync.dma_start(
            out=wk_sb, in_=wk_d[:, :].rearrange("(c p) m -> p c m", p=P)
        )
        nc.sync.dma_start(
            out=wq_sb, in_=wq_d[:, :].rearrange("(c p) m -> p c m", p=P)
        )
        nc.sync.dma_start(
            out=wv_sb, in_=wv_d[:, :].rearrange("(c p) m -> p c m", p=P)
        )
        qb_t = singles.tile([P, 1], f32)
        kb_t = singles.tile([P, 1], f32)
        vb_t = singles.tile([P, 1], f32)
        bo_t = singles.tile([P, 1], f32)
        for b_t, b_d in ((qb_t, qb_d), (kb_t, kb_d), (vb_t, vb_d), (bo_t, bo_d)):
            nc.sync.dma_start(out=b_t, in_=b_d[:].rearrange("(p o) -> p o", o=1))

        # long-lived activations
        qT = singles.tile([P, N], dt_mm)        # [2*64 qdims, rows]
        kT = singles.tile([P, N], dt_mm)
        v_sb = singles.tile([P, RT, HL, DH + 1], dt_mm)  # [key%128, kc, h, v|1]
        attn_h = [singles.tile([DH, N], dt_mm, name=f"attn_h{h}")
                  for h in range(HL)]
        bo_row = singles.tile([4, P], dt_mm)     # row 0 = b_out slice as a row

        nc.gpsimd.memset(v_sb[:, :, :, DH:], 1.0)  # ones column

        # ---- stages A-C: LayerNorm -> xn^T -> q/k/v ------------------------
        def dummy_mms(n, width=256):
            for _ in range(n):
                dmt = dump.tile([P, RC_W], f32, tag="warm")
                nc.tensor.matmul(dmt[:, 0:width], ident,
                                 warm_rhs[:, 0:width], start=True,
                                 stop=True)

        with (
            tc.tile_pool(name="stat", bufs=4) as statp,
            tc.tile_pool(name="tp", bufs=2, space="PSUM") as tp,
            tc.tile_pool(name="mmp", bufs=2, space="PSUM") as mmp,
            tc.tile_pool(name="ptm", bufs=1, space="PSUM") as ptmp,
            tc.tile_pool(name="xnTp", bufs=1) as xnTp,
        ):
            xnT = xnTp.tile([P, DC, N], dt_mm)   # [dim%128, dimchunk, rows]
            vT = xnTp.tile([P, N], dt_mm)

            # dependency-free burst to cover the first x DMA + LN latency
            dummy_mms(8)

            # b_out as a row (for the projection's rank-1 bias)
            bo_bf = xnTp.tile([P, 1], dt_mm)
            with nc.allow_low_precision(reason="bias cast"):
                nc.vector.tensor_copy(out=bo_bf, in_=bo_t)
            bp = ptmp.tile([4, P], f32, tag="ptm")
            nc.tensor.matmul(bp[0:1, 0:P], bo_bf[:, 0:1], ident,
                             start=True, stop=True)
            with nc.allow_low_precision(reason="bias row"):
                nc.vector.tensor_copy(out=bo_row[0:1, :], in_=bp[0:1, 0:P])

            for g4 in range(RT // 4):
                for j in range(4):
                    rt = g4 * 4 + j
                    if rt < RT - 3:
                        nx = xp.tile([P, D], f32, tag="x", name=f"x{rt + 3}")
                        nc.sync.dma_start(
                            out=nx, in_=x_d[(rt + 3) * P:(rt + 4) * P, :]
                        )
                        x_tiles.append(nx)
                    x_t = x_tiles[rt]
                    stats = statp.tile([P, 2, 6], f32, tag="st")
                    for sg in range(2):
                        nc.vector.bn_stats(
                            out=stats[:, sg, :],
                            in_=x_t[:, sg * 512:(sg + 1) * 512],
                        )
                    mv = statp.tile([P, 2], f32, tag=f"mv{j}")
                    nc.vector.bn_aggr(out=mv, in_=stats)
                    rstd = statp.tile([P, 1], f32, tag=f"rstd{j}")
                    nc.scalar.activation(
                        out=rstd, in_=mv[:, 1:2], func=AF.Sqrt,
                        bias=eps_t, scale=1.0,
                    )
                    nc.vector.reciprocal(out=rstd, in_=rstd)
                    xh_t = xhp.tile([P, D], dt_mm, tag="xh")
                    nc.vector.tensor_scalar(
                        out=xh_t, in0=x_t,
                        scalar1=mv[:, 0:1], scalar2=rstd,
                        op0=mybir.AluOpType.subtract,
                        op1=mybir.AluOpType.mult,
                    )
                    x_tiles[rt] = xh_t
                if g4 == 0:
                    nc.sync.dma_start(
                        out=wo_sb,
                        in_=wo_d[:, :].rearrange("(c p) m -> p c m", p=P),
                    )

                for j in range(4):
                    rt = g4 * 4 + j
                    xh_t = x_tiles[rt]
                    for g in range(2):
                        pt = tp.tile([P, 512], f32, tag="pt")
                        for jj in range(4):
                            dc = g * 4 + jj
                            nc.tensor.matmul(
                                pt[:, jj * P:(jj + 1) * P],
                                xh_t[:, dc * P:(dc + 1) * P],
                                ident, start=True, stop=True,
                            )
                        dst = xnT[:, g * 4:(g + 1) * 4, rt * P:(rt + 1) * P]
                        tsrc = pt[:].rearrange("p (jj q) -> p jj q", jj=4)
                        with nc.allow_low_precision(reason="transpose evac"):
                            if (rt + 2 * g) % 4 == 0:
                                nc.vector.tensor_copy(out=dst, in_=tsrc)
                            else:
                                nc.scalar.copy(out=dst, in_=tsrc)
                    dummy_mms(1)

                # q/k/v projections for this 512-row block
                nt = g4
                for w_sb, b_t, dst in (
                    (wk_sb, kb_t, kT), (wq_sb, qb_t, qT), (wv_sb, vb_t, vT),
                ):
                    pm = mmp.tile([P, 512], f32, tag="pm")
                    for kc in range(DC):
                        nc.tensor.matmul(
                            pm,
                            w_sb[:, kc, :],
                            xnT[:, kc, nt * 512:(nt + 1) * 512],
                            start=(kc == 0), stop=(kc == DC - 1),
                        )
                    nc.scalar.activation(
                        out=dst[:, nt * 512:(nt + 1) * 512], in_=pm,
                        func=AF.Identity, bias=b_t, scale=1.0,
                    )
                # v^T -> v (row-major with ones column) for this block
                for rt in range(g4 * 4, g4 * 4 + 4):
                    pt = tp.tile([P, 512], f32, tag="pt")
                    nc.tensor.matmul(
                        pt[:, :P], vT[:, rt * P:(rt + 1) * P], ident,
                        start=True, stop=True,
                    )
                    with nc.allow_low_precision(reason="v evac"):
                        nc.vector.tensor_copy(
                            out=v_sb[:, rt, :, 0:DH],
                            in_=pt[:, :P].rearrange("p (h d) -> p h d", h=HL),
                        )
                dummy_mms(1)

        # ---- stage D: attention, one continuous global kc stream -----------
        with (
            tc.tile_pool(name="expp", bufs=2) as expp,
            tc.tile_pool(name="rsum", bufs=2) as rsump,
            tc.tile_pool(name="sp", bufs=2, space="PSUM") as sp,
            tc.tile_pool(name="op", bufs=1, space="PSUM") as op,
            tc.tile_pool(name="agp", bufs=2) as agp,
        ):
            state = {}

            def sim_group(s, kc):
                r0, w = CHUNKS[s]
                st = state[s]
                ps = sp.tile([P, 2 * RC_W], f32, tag="ps", name=f"ps{s}_{kc}")
                for h in range(HL):
                    nc.tensor.matmul(
                        ps[:, h * RC_W:h * RC_W + w],
                        kT[h * DH:(h + 1) * DH, kc * P:(kc + 1) * P],
                        qT[h * DH:(h + 1) * DH, r0:r0 + w],
                        start=True, stop=True,
                    )
                nc.scalar.activation(
                    out=st["exp_t"][:, kc, :, 0:w],
                    in_=ps[:].rearrange("p (c q) -> p c q", c=2)[:, :, 0:w],
                    func=AF.Exp, scale=SIM_SCALE,
                )

            def av_pair(s, kc):
                _, w = CHUNKS[s]
                st = state[s]
                if st["po"] is None:
                    st["po"] = op.tile([P, 2 * RC_W], f32, tag="po",
                                       name=f"po{s}")
                for h in range(HL):
                    nc.tensor.matmul(
                        st["po"][0:DH + 1, h * RC_W:h * RC_W + w],
                        v_sb[:, kc, h, :],
                        st["exp_t"][:, kc, h, 0:w],
                        start=(kc == 0), stop=(kc == RT - 1),
                    )

            def norm_chain(s):
                """Denominators -> reciprocals -> normalize -> AllGather."""
                r0, w = CHUNKS[s]
                st = state[s]
                den = rsump.tile([DH + 1, 2 * RC_W], dt_mm, tag="den",
                                 name=f"den{s}")
                with nc.allow_low_precision(reason="softmax denom"):
                    nc.vector.tensor_copy(
                        out=den[DH:DH + 1, :], in_=st["po"][DH:DH + 1, :]
                    )
                pr = sp.tile([P, 2 * RC_W], f32, tag="ps", name=f"pr{s}")
                for h in range(HL):
                    nc.tensor.matmul(
                        pr[0:DH, h * RC_W:h * RC_W + w],
                        ones_row[DH:DH + 1, 0:DH],
                        den[DH:DH + 1, h * RC_W:h * RC_W + w],
                        start=True, stop=True,
                    )
                rb = rsump.tile([DH, 2 * RC_W], f32, tag="rb", name=f"rb{s}")
                for h in range(HL):
                    nc.vector.reciprocal(
                        out=rb[:, h * RC_W:h * RC_W + w],
                        in_=pr[0:DH, h * RC_W:h * RC_W + w],
                    )
                    with nc.allow_low_precision(reason="attn bf16 wire"):
                        nc.vector.tensor_tensor(
                            out=attn_h[h][:, r0:r0 + w],
                            in0=st["po"][0:DH, h * RC_W:h * RC_W + w],
                            in1=rb[:, h * RC_W:h * RC_W + w],
                            op=mybir.AluOpType.mult,
                        )
                    nc.sync.dma_start(
                        out=ag_in[s][h * DH:(h + 1) * DH, :],
                        in_=attn_h[h][:, r0:r0 + w],
                    )
                nc.gpsimd.collective_compute(
                    "AllGather",
                    mybir.AluOpType.bypass,
                    replica_groups=groups,
                    ins=[ag_in[s][:].opt()],
                    outs=[ag_out[s][:].opt()],
                )
                if DEBUG:
                    st["den_t"] = den

            def proj_dma(s):
                """Gathered-heads load: one strided DMA, issued off the ACT
                hwdge path so AG-waits never block the SP store queue."""
                _, w = CHUNKS[s]
                st = state[s]
                agt = agp.tile([P, DC, RC_W], dt_mm, tag="agt", name=f"agt{s}")
                st["agt"] = agt
                nc.scalar.dma_start(
                    out=agt[:, :, 0:w],
                    in_=ag_out[s][:, :].rearrange("(c p) q -> p c q", p=P),
                )

            def proj(s):
                r0, w = CHUNKS[s]
                st = state[s]
                agt = st["agt"]
                pf = dump.tile([P, RC_W], f32, tag="warm", name=f"pf{s}")
                for kc in range(DC):
                    nc.tensor.matmul(
                        pf[:, 0:w], wo_sb[:, kc, :], agt[:, kc, 0:w],
                        start=(kc == 0), stop=False,
                    )
                nc.tensor.matmul(
                    pf[:, 0:w], bo_row[0:1, :], ones_row[0:1, 0:w],
                    start=False, stop=True,
                )
                ot = agp.tile([P, RC_W], f32, tag="ot", name=f"ot{s}")
                nc.vector.tensor_copy(out=ot[:, 0:w], in_=pf[:, 0:w])
                nc.sync.dma_start(out=out_d[:, r0:r0 + w], in_=ot[:, 0:w])

            for s in range(S):
                state[s] = {
                    "exp_t": expp.tile([P, RT, HL, RC_W], dt_mm, tag="exp",
                                       name=f"exp{s}"),
                    "po": None, "agt": None,
                }

            # av catch-up schedule: stage s's first 6 avs run two-per-slot at
            # slots 16s+8..10 (so po(s-1) -> po(s) hand-off has time), the
            # rest trail sim by 5 slots; av(s,15) lands at slot 16s+20.
            av_at = {}
            for s in range(S):
                for kc in range(RT):
                    slot = s * RT + (8 + kc // 2 if kc < 6 else 5 + kc)
                    av_at.setdefault(slot, []).append((s, kc))

            total = S * RT + RT
            for t in range(total):
                s, kc = divmod(t, RT)
                if t < S * RT:
                    sim_group(s, kc)
                    dummy_mms(1)
                for us, ukc in av_at.get(t, ()):
                    av_pair(us, ukc)
                    if ukc == RT - 1:
                        norm_chain(us)
                if t % RT == 2 and 3 <= t // RT < S:
                    proj_dma(t // RT - 3)
                if t % RT == 8 and 3 <= t // RT < S:
                    proj(t // RT - 3)
            # drain: remaining projections (these may wait on the last AGs)
            for s in range(max(0, S - 3), S):
                proj_dma(s)
                proj(s)

            if DEBUG:
                dbg_sb = agp.tile([P, N + 128], f32, tag="dbg")
                for src, dst in ((qT, dbg_qt), (kT, dbg_kt)):
                    nc.vector.tensor_copy(out=dbg_sb[:, 0:N], in_=src)
                    nc.sync.dma_start(out=dst[:, :], in_=dbg_sb[:, 0:N])
                nc.vector.tensor_copy(
                    out=dbg_sb[:, 0:RT * HL * (DH + 1)],
                    in_=v_sb[:].rearrange("p a b c -> p (a b c)"),
                )
                nc.sync.dma_start(out=dbg_v[:, :],
                                  in_=dbg_sb[:, 0:RT * HL * (DH + 1)])
                nc.vector.tensor_copy(out=dbg_sb[0:DH + 1, 0:2 * RC_W],
                                      in_=state[0]["den_t"])
                nc.sync.dma_start(out=dbg_den[:, :],
                                  in_=dbg_sb[0:DH + 1, 0:2 * RC_W])
                nc.vector.tensor_copy(out=dbg_sb[0:DH, 0:N], in_=attn_h[0])
                nc.sync.dma_start(out=dbg_attn[:, :], in_=dbg_sb[0:DH, 0:N])

    if not nc.is_finalized():
        nc.finalize()
    return nc


def _get_built():
    global _BUILT
    if _BUILT is None:
        _BUILT = _build()
    return _BUILT


def _shard_inputs(x, ln_scale, ln_bias, w_qkv, w_out, b_out):
    """Host-side sharding: slice per-head weight columns, fold LN params."""
    x = np.ascontiguousarray(np.asarray(x, np.float32))
    ln_scale = np.asarray(ln_scale, np.float32)
    ln_bias = np.asarray(ln_bias, np.float32)
    w_qkv = np.asarray(w_qkv, np.float32)
    w_out = np.asarray(w_out, np.float32)
    b_out = np.asarray(b_out, np.float32)

    w_np = {"f32": np.float32, "f32r": np.float32,
            "bf16": ml_dtypes.bfloat16}[MM_DT]

    in_maps = []
    for ci in range(NCORES):
        c0 = ci * HC
        sl = {}
        for name, off in (("q", 0), ("k", HEADS * DH), ("v", 2 * HEADS * DH)):
            w = w_qkv[:, off + c0: off + c0 + HC]
            sl["w" + name] = np.ascontiguousarray(
                (ln_scale[:, None] * w).astype(w_np)
            )
            sl[name + "b"] = np.ascontiguousarray(ln_bias @ w)
        sl["wo"] = np.ascontiguousarray(w_out[:, c0:c0 + HC].astype(w_np))
        sl["bo"] = np.ascontiguousarray(b_out[c0:c0 + HC])
        sl["x"] = x
        in_maps.append(sl)
    return in_maps


def kernel(x, ln_scale, ln_bias, w_qkv, w_out, b_out):
    from concourse.bass_utils import run_bass_kernel_spmd

    nc = _get_built()
    in_maps = _shard_inputs(x, ln_scale, ln_bias, w_qkv, w_out, b_out)
    res = run_bass_kernel_spmd(nc, in_maps, core_ids=list(range(NCORES)))
    shards = [res.results[ci]["out"] for ci in range(NCORES)]  # [128, 2048] each
    outT = np.concatenate(shards, axis=0)  # [1024, 2048]
    return np.ascontiguousarray(outT.T)
